# revision 45
# baseline (speedup 1.0000x reference)
"""Causal self-attention kernel for Trainium2, 8 NeuronCores.

Problem: B=4, T=2048, C=1024, 16 heads, D=64 (fp32).
Sharding: core i handles batch b=i//2 and head-group hg=i%2 (8 heads each).
Each core computes qkv + attention + its partial projection; the host sums
the two head-group partials per batch and adds b_proj.

Dataflow (all matmul operands bf16, fp32 PSUM accumulate; rel err 3.6e-3):
  Wavefront (per 128-row t-chunk, DMA loads explicitly sequenced on the sync
    queue): x DMA (bf16, host-cast) -> PE transpose (1c/row) -> XT;
    V = x@Wv -> VA; QK(g=0) per 512-col strip.
  VA holds [ones x64 | V_h] per head, so each PV matmul (cost = N only)
    also produces the softmax denominators replicated on PSUM partitions
    0-63 while O lands on 64-127: normalization is a single DVE reciprocal
    + multiply (PSUM/SBUF operands may use different base partitions).
  ATT(g): scores (2 heads row-packed at tile_position 64), exp on Act,
    causal masks on gpsimd, PV; QK(g+1) pieces and (at g=3) the projection
    are interleaved between strips as PE filler, front-loaded at q-block
    boundaries where the next PV waits on the psO slot.  PV emission lags
    scores by 2 strip-pairs to avoid head-of-line blocking.
  Tail: warmup matmuls hold the PE p-state through the final normalize
    chain so the last projection block runs at full clock; outputs are
    stored bf16 (host upcasts and sums the partials).
"""

import numpy as np

N_CORES = 8
T = 2048
C = 1024
HL = 8          # heads per core
D = 64
KC = C // 128   # 8 contraction chunks
NT = T // 128   # 16 t-tiles
NQ = T // 512   # 4 q-tiles
VW = HL * 128   # 1024 v-aug cols per t-tile ([ones x64 | V] per head)

_CACHE = {}


def _build(phases=('A', 'QK', 'ATT', 'D')):
    from contextlib import ExitStack
    import concourse.bass as bass
    from concourse import bacc
    import concourse.mybir as mybir
    import concourse.tile as tile
    from concourse.masks import make_identity

    F32 = mybir.dt.float32
    BF16 = mybir.dt.bfloat16
    EXP = mybir.ActivationFunctionType.Exp
    ISGE = mybir.AluOpType.is_ge
    W15 = C + C // 2  # 1536

    nc = bacc.Bacc("TRN2", target_bir_lowering=False, debug=False,
                   num_devices=N_CORES)

    x_d = nc.dram_tensor("x", [T, C], BF16, kind="ExternalInput")
    wqkv_d = nc.dram_tensor("w_qkv", [C, W15], BF16, kind="ExternalInput")
    wproj_d = nc.dram_tensor("w_proj", [512, C], BF16, kind="ExternalInput")
    bqk_d = nc.dram_tensor("b_qk", [128, 8], F32, kind="ExternalInput")
    bv_d = nc.dram_tensor("b_v", [128, 512], F32, kind="ExternalInput")
    ones_d = nc.dram_tensor("ones64", [128, 128], BF16, kind="ExternalInput")
    out_d = nc.dram_tensor("out", [T, C], BF16, kind="ExternalOutput")
    if 'DBG' in phases:
        xt_dbg = nc.dram_tensor("xt_dbg", [128, KC * T], BF16, kind="ExternalOutput")
        va_dbg = nc.dram_tensor("va_dbg", [128, NT * VW], BF16, kind="ExternalOutput")
        qkt_dbg = nc.dram_tensor("qkt_dbg", [4, 128, 2 * T], BF16, kind="ExternalOutput")
        ot_dbg = nc.dram_tensor("ot_dbg", [128, 4 * T], BF16, kind="ExternalOutput")

    with tile.TileContext(nc) as tc, ExitStack() as ctx:
        # ---------- persistent pools ----------
        consts = ctx.enter_context(tc.tile_pool(name="consts", bufs=1))
        big = ctx.enter_context(tc.tile_pool(name="big", bufs=1))
        wqkp = ctx.enter_context(tc.tile_pool(name="wqk", bufs=2))
        qktp = ctx.enter_context(tc.tile_pool(name="qkt", bufs=2))

        ident = consts.tile([128, 128], BF16)
        make_identity(nc, ident[:])
        bqk_sb = consts.tile([128, 8], F32)
        nc.gpsimd.dma_start(out=bqk_sb[:], in_=bqk_d[:])
        ones_sb = consts.tile([128, 128], BF16)
        nc.gpsimd.dma_start(out=ones_sb[:], in_=ones_d[:])
        bv_sb = consts.tile([128, 512], F32)
        nc.gpsimd.dma_start(out=bv_sb[:], in_=bv_d[:])

        XT = big.tile([128, KC * T], BF16)        # 32 KB/part, x transposed
        VA = big.tile([128, NT * VW], BF16)       # 16.25 KB/part, v-aug
        OT = big.tile([128, 4 * T], BF16)         # 16 KB/part, attn out^T

        # ones-block columns of VA (cols [0:64) of each 128-col head group)
        va_ones = VA[:].rearrange("p (i c) -> p i c", c=128)[:, :, 0:64]
        nc.vector.tensor_copy(
            va_ones,
            ones_sb[:, 0:64].rearrange("p (x c) -> p x c", x=1)
            .broadcast_to([128, NT * HL, 64]))

        def emit_wqk(g, eng, halves=(0, 1)):
            wqk = wqkp.tile([128, 2 * KC * 128], BF16, tag="wqk", name=f"wqk{g}")
            for half in halves:
                eng.dma_start(
                    out=wqk[:, half * KC * 128:(half + 1) * KC * 128]
                        .rearrange("p (k m) -> p k m", k=KC),
                    in_=wqkv_d[:, half * 512 + g * 128: half * 512 + (g + 1) * 128]
                        .rearrange("(k p) m -> p k m", p=128),
                )
            return wqk

        def emit_wqk_half(g, wqk, half, eng):
            eng.dma_start(
                out=wqk[:, half * KC * 128:(half + 1) * KC * 128]
                    .rearrange("p (k m) -> p k m", k=KC),
                in_=wqkv_d[:, half * 512 + g * 128: half * 512 + (g + 1) * 128]
                    .rearrange("(k p) m -> p k m", p=128),
            )

        def emit_qk_chunk(g, wqk, qkt, nt4, psp):
            # QK(g) for t-strip nt4 (512 cols), both halves -> qkt bf16
            for half in (0, 1):
                pqk = psp.tile([128, 512], F32, tag="mm")
                for k in range(KC):
                    nc.tensor.matmul(
                        pqk[:],
                        wqk[:, half * KC * 128 + k * 128:
                               half * KC * 128 + (k + 1) * 128],
                        XT[:, k * T + nt4 * 512: k * T + (nt4 + 1) * 512],
                        start=(k == 0), stop=(k == KC - 1))
                nc.vector.tensor_scalar_add(
                    qkt[:, half * T + nt4 * 512: half * T + (nt4 + 1) * 512],
                    pqk[:],
                    bqk_sb[:, half * 4 + g: half * 4 + g + 1])

        def qk_chunk_steps(g, wqk, qkt, nt4, psp):
            # Same work as emit_qk_chunk but as a list of thunks, each
            # emitting a 2-matmul piece, so QK(g+1) can be spread between
            # attention strips (fills PE while Act works through the exps).
            steps = []
            state = {}
            def make_step(half, k2):
                def step():
                    if k2 == 0:
                        state[half] = psp.tile([128, 512], F32, tag="mm",
                                               name=f"pqk{g}_{nt4}_{half}")
                    pqk = state[half]
                    for k in (2 * k2, 2 * k2 + 1):
                        nc.tensor.matmul(
                            pqk[:],
                            wqk[:, half * KC * 128 + k * 128:
                                   half * KC * 128 + (k + 1) * 128],
                            XT[:, k * T + nt4 * 512: k * T + (nt4 + 1) * 512],
                            start=(k == 0), stop=(k == KC - 1))
                    if k2 == KC // 2 - 1:
                        nc.vector.tensor_scalar_add(
                            qkt[:, half * T + nt4 * 512:
                                half * T + (nt4 + 1) * 512],
                            pqk[:],
                            bqk_sb[:, half * 4 + g: half * 4 + g + 1])
                return step
            for half in (0, 1):
                for k2 in range(KC // 2):
                    steps.append(make_step(half, k2))
            return steps

        # ---------- wavefront: transpose x + V + QK(g=0) ----------
        # All DRAM loads go on the sync queue in an explicit order matched to
        # PE demand (the cost model's DMA pipe is serialized at ~350 B/ns):
        # x0ab, wv0, x1, wv1, x2, x3, wqk0a, x4, wqk0b, x5..x15, wqk1.
        # V(it) is split k0-3/k4-7 around transposes(it+1) so its first half
        # runs as soon as the first wv half lands.
        wqks = {}
        qkts = {}
        if 'A' in phases:
          with (
            tc.tile_pool(name="xnat", bufs=5) as xnat,
            tc.tile_pool(name="wv", bufs=1) as wvp,
            tc.tile_pool(name="psT", bufs=3, space="PSUM") as psT,
            tc.tile_pool(name="psA", bufs=2, space="PSUM") as psA,
          ):
            wv = wvp.tile([128, KC * 512], BF16)
            if 'QK' in phases:
                qkts[0] = qktp.tile([128, 2 * T], BF16, tag="qkt", name="qkt0")

            def emit_wv_half(h):
                nc.sync.dma_start(
                    out=wv[:, h * 4 * 512:(h + 1) * 4 * 512]
                        .rearrange("p (k m) -> p k m", k=4),
                    in_=wqkv_d[512 * h: 512 * (h + 1), 1024:1536]
                        .rearrange("(k p) m -> p k m", p=128),
                )

            pvs = {}

            def emit_v_part(it, part):
                if part == 0:
                    pvs[it] = psA.tile([128, 512], F32, tag="pv",
                                       name=f"pv{it}")
                pv = pvs[it]
                for k in range(4 * part, 4 * part + 4):
                    nc.tensor.matmul(
                        pv[:],
                        XT[:, k * T + it * 128: k * T + (it + 1) * 128],
                        wv[:, k * 512:(k + 1) * 512],
                        start=(k == 0), stop=(k == KC - 1))
                if part == 1:
                    va_dst = VA[:, it * VW:(it + 1) * VW].rearrange(
                        "p (h c) -> p h c", h=HL)[:, :, 64:128]
                    nc.vector.tensor_add(
                        va_dst,
                        pv[:].rearrange("p (h c) -> p h c", h=HL),
                        bv_sb[:].rearrange("p (h c) -> p h c", h=HL))

            for it in range(NT):
                xt = xnat.tile([128, C], BF16, tag="xn")
                if it == 0:
                    for hc in (0, 1):
                        nc.sync.dma_start(
                            out=xt[:, hc * 512:(hc + 1) * 512],
                            in_=x_d[0:128, hc * 512:(hc + 1) * 512])
                else:
                    nc.sync.dma_start(out=xt[:],
                                      in_=x_d[it * 128:(it + 1) * 128, :])
                if it == 1:
                    emit_wv_half(0)
                    emit_wv_half(1)
                elif 'QK' in phases and it == 4:
                    wqks[0] = emit_wqk(0, nc.sync, halves=(0,))
                elif 'QK' in phases and it == 5:
                    emit_wqk_half(0, wqks[0], 1, nc.sync)
                # V(it-1) part A: emitted after this iteration's weight DMAs
                # (its wv reads must follow the wv writes in program order)
                if it > 0:
                    emit_v_part(it - 1, 0)
                for c2 in (0, 1):
                    pt = psT.tile([128, 512], BF16, tag="tp")
                    for j in range(4):
                        nc.tensor.transpose(
                            pt[:, j * 128:(j + 1) * 128],
                            xt[:, (4 * c2 + j) * 128:(4 * c2 + j + 1) * 128],
                            ident[:])
                    dst = XT[:].rearrange("p (k t) -> p k t", k=KC)[
                        :, 4 * c2:4 * c2 + 4, it * 128:(it + 1) * 128]
                    src = pt[:].rearrange("p (k t) -> p k t", k=4)
                    if c2 == 0:
                        nc.vector.tensor_copy(dst, src)
                    else:
                        nc.scalar.copy(dst, src)
                if it > 0:
                    emit_v_part(it - 1, 1)
                if 'QK' in phases and it % 4 == 3 and it > 3:
                    emit_qk_chunk(0, wqks[0], qkts[0], it // 4 - 1, psA)
            emit_v_part(NT - 1, 0)
            emit_v_part(NT - 1, 1)
            if 'QK' in phases:
                emit_qk_chunk(0, wqks[0], qkts[0], NQ - 1, psA)
                wqks[1] = emit_wqk(1, nc.sync)

        # ---------- ATT(g) with QK(g+1) + D interleaved ----------
        if 'QK' in phases and 'ATT' in phases:
          with (
            tc.tile_pool(name="ptile", bufs=5) as ptp,
            tc.tile_pool(name="rsc", bufs=1) as rscp,
            tc.tile_pool(name="wp", bufs=1) as wpp,
            tc.tile_pool(name="stage", bufs=3) as stagep,
            tc.tile_pool(name="dpart", bufs=2) as dpartp,
            tc.tile_pool(name="psmm", bufs=2, space="PSUM") as psmm,
            tc.tile_pool(name="psS", bufs=2, space="PSUM") as psS,
            tc.tile_pool(name="psO", bufs=1, space="PSUM") as psO,
          ):
            wp = wpp.tile([128, 4 * C], BF16)
            if 'D' in phases:
                nc.gpsimd.dma_start(
                    out=wp[:].rearrange("p (g m) -> p g m", g=4),
                    in_=wproj_d[:].rearrange("(g p) m -> p g m", p=128),
                )
            for g in range(4):
                if g + 2 < 4:
                    wqks[g + 2] = emit_wqk(g + 2, nc.gpsimd)
                qkt = qkts[g]
                if g + 1 < 4:
                    qkts[g + 1] = qktp.tile([128, 2 * T], BF16, tag="qkt",
                                            name=f"qkt{g+1}")
                dparts = {}

                def d_partial_steps(qt):
                    # gg=0..2 projection partials for q-block qt: OT strips
                    # for head-groups 0-2 are complete once g=2 finished, so
                    # these can run as PE filler during ATT(3, qt) itself
                    steps = []
                    def make_step(it, n):
                        def step():
                            part = dpartp.tile([128, 512], F32,
                                               tag=f"part{it % 4}_{n}",
                                               name=f"part{it}_{n}")
                            dparts[(it, n)] = part
                            pp = psmm.tile([128, 512], F32, tag="mm",
                                           name=f"ppa{it}_{n}")
                            for gg in range(3):
                                nc.tensor.matmul(
                                    pp[:],
                                    OT[:, gg * T + it * 128: gg * T + (it + 1) * 128],
                                    wp[:, gg * C + n * 512: gg * C + (n + 1) * 512],
                                    start=(gg == 0), stop=(gg == 2))
                            nc.vector.tensor_copy(part[:], pp[:])
                        return step
                    for it in range(4 * qt, 4 * qt + 4):
                        for n in (0, 1):
                            steps.append(make_step(it, n))
                    return steps

                def d_steps(qt):
                    # whole-group projection (used for the tail block where
                    # the serial DVE adds of the split form would bind)
                    steps = []
                    state = {}
                    def make_step(it, n):
                        def step():
                            if n == 0:
                                state[it] = stagep.tile([128, C], BF16,
                                                        tag="stg",
                                                        name=f"stg{it}")
                            stage = state[it]
                            pp = psmm.tile([128, 512], F32, tag="mm",
                                           name=f"pp{it}_{n}")
                            for gg in range(4):
                                nc.tensor.matmul(
                                    pp[:],
                                    OT[:, gg * T + it * 128: gg * T + (it + 1) * 128],
                                    wp[:, gg * C + n * 512: gg * C + (n + 1) * 512],
                                    start=(gg == 0), stop=(gg == 3))
                            if n == 0:
                                nc.vector.tensor_copy(
                                    stage[:, 0:512], pp[:])
                            else:
                                nc.scalar.copy(
                                    stage[:, 512:1024], pp[:])
                            nc.sync.dma_start(
                                out=out_d[it * 128:(it + 1) * 128,
                                          n * 512:(n + 1) * 512],
                                in_=stage[:, n * 512:(n + 1) * 512])
                        return step
                    for it in range(4 * qt, 4 * qt + 4):
                        for n in (0, 1):
                            steps.append(make_step(it, n))
                    return steps

                def d_complete_steps(qt):
                    # gg=3 + add partial + store, per (it, n)
                    steps = []
                    state = {}
                    def make_step(it, n):
                        def step():
                            if n == 0:
                                state[it] = stagep.tile([128, C], BF16,
                                                        tag="stg",
                                                        name=f"stg{it}")
                            stage = state[it]
                            pp = psmm.tile([128, 512], F32, tag="mm",
                                           name=f"ppb{it}_{n}")
                            nc.tensor.matmul(
                                pp[:],
                                OT[:, 3 * T + it * 128: 3 * T + (it + 1) * 128],
                                wp[:, 3 * C + n * 512: 3 * C + (n + 1) * 512],
                                start=True, stop=True)
                            nc.vector.tensor_add(
                                stage[:, n * 512:(n + 1) * 512], pp[:],
                                dparts[(it, n)][:])
                            nc.sync.dma_start(
                                out=out_d[it * 128:(it + 1) * 128,
                                          n * 512:(n + 1) * 512],
                                in_=stage[:, n * 512:(n + 1) * 512])
                        return step
                    for it in range(4 * qt, 4 * qt + 4):
                        for n in (0, 1):
                            steps.append(make_step(it, n))
                    return steps

                if 'DBG' in phases:
                    nc.scalar.dma_start(out=qkt_dbg[g], in_=qkt[:])
                for qt in range(NQ):
                    qk_steps = []
                    if g + 1 < 4:
                        qk_steps = qk_chunk_steps(g + 1, wqks[g + 1],
                                                  qkts[g + 1], qt, psmm)
                    elif 'D' in phases:
                        if qt > 0:
                            qk_steps = d_complete_steps(qt - 1)
                        if qt < 3:
                            qk_steps = qk_steps + d_partial_steps(qt)
                    nsteps = len(qk_steps)
                    nstrips = 2 * (2 * qt + 2)
                    popped = [0]

                    def pop_qk(frac):
                        want = int(round(frac * nsteps))
                        while popped[0] < want:
                            qk_steps[popped[0]]()
                            popped[0] += 1
                    strip_i = [0]
                    psO0 = psO.tile([128, 512], F32, tag="o0")
                    psO1 = psO.tile([128, 512], F32, tag="o1")
                    psOh = [psO0, psO1]
                    if nsteps:
                        # front-load filler into the qt-boundary window where
                        # the PE would otherwise stall on the psO slot
                        pop_qk(0.25)

                    jlast = 4 * qt + 3
                    # q-restriction per diagonal delta: computed q-range
                    # [qoff, 512); bf16 matmuls run 1c/row at any N so the
                    # delta-3 strip computes only its 128-col triangle block.
                    QOFF = (0, 128, 256, 384)

                    def emit_pv(s, hi, ptile):
                        diag = s >= 2 * qt
                        h = 2 * g + hi
                        strip_i[0] += 1
                        if nsteps:
                            pop_qk(0.25 + 0.75 * strip_i[0] / nstrips)
                        for dd in (0, 1):
                            j = 2 * s + dd
                            qoff = QOFF[j - 4 * qt] if diag else 0
                            nc.tensor.matmul(
                                psOh[hi][:, qoff:512],
                                VA[:, j * VW + h * 128: j * VW + (h + 1) * 128],
                                ptile[:, dd * 512 + qoff:(dd + 1) * 512],
                                start=(j == 0), stop=(j == jlast))

                    # PV emission lags scores by 2 strip-pairs so the PE has
                    # score work queued while the previous q-block's psO is
                    # still being normalized (avoids head-of-line blocking).
                    pend = []
                    for s in range(2 * qt + 2):
                        diag = s >= 2 * qt
                        for hi in (0, 1):
                            psSt = psS.tile([128, 1024], F32, tag="psS")
                            for dd in (0, 1):
                                j = 2 * s + dd
                                qoff = QOFF[j - 4 * qt] if diag else 0
                                nc.tensor.matmul(
                                    psSt[:, dd * 512 + qoff:(dd + 1) * 512],
                                    qkt[64 * hi:64 * hi + 64,
                                        T + j * 128: T + (j + 1) * 128],
                                    qkt[64 * hi:64 * hi + 64,
                                        qt * 512 + qoff:(qt + 1) * 512],
                                    start=True, stop=True,
                                    tile_position=(64 * hi, 0))
                            ptile = ptp.tile([128, 1024], BF16, tag=f"pt{hi}")
                            if diag and s == 2 * qt + 1:
                                # deltas 2,3: cols [256:512] and [896:1024]
                                nc.scalar.activation(
                                    ptile[:, 256:512], psSt[:, 256:512],
                                    EXP, scale=0.125)
                                nc.scalar.activation(
                                    ptile[:, 896:1024], psSt[:, 896:1024],
                                    EXP, scale=0.125)
                            else:
                                nc.scalar.activation(ptile[:], psSt[:], EXP,
                                                     scale=0.125)
                            if diag:
                                for dd in (0, 1):
                                    delta = 2 * (s - 2 * qt) + dd
                                    # triangle block at cols [128*delta,+128):
                                    # keep where (q rel block) - k >= 0
                                    sl = slice(dd * 512 + 128 * delta,
                                               dd * 512 + 128 * delta + 128)
                                    nc.gpsimd.affine_select(
                                        out=ptile[:, sl], in_=ptile[:, sl],
                                        compare_op=ISGE, fill=0.0, base=0,
                                        pattern=[[1, 128]],
                                        channel_multiplier=-1)
                            pend.append((s, hi, ptile))
                            if len(pend) > 6:
                                emit_pv(*pend.pop(0))
                    for item in pend:
                        emit_pv(*item)
                    # normalize + store OT: sums sit replicated on psO
                    # partitions 0-63, O on 64-127 (PSUM/SBUF operands may
                    # use different base partitions)
                    for hi in (0, 1):
                        bc_sb = rscp.tile([64, 512], F32, tag=f"bc{hi}")
                        with nc.allow_low_precision(reason="softmax recip"):
                            nc.vector.reciprocal(bc_sb[:], psOh[hi][0:64, :])
                        if hi == 0:
                            nc.vector.tensor_mul(
                                OT[0:64, g * T + qt * 512: g * T + (qt + 1) * 512],
                                psOh[0][64:128, :], bc_sb[:])
                        else:
                            otmp = rscp.tile([64, 512], BF16, tag="otmp")
                            nc.vector.tensor_mul(otmp[:], psOh[1][64:128, :],
                                                 bc_sb[:])
                            qeng = nc.scalar if (g == 3 and qt == 3) else nc.sync
                            qeng.dma_start(
                                out=OT[64:128, g * T + qt * 512: g * T + (qt + 1) * 512],
                                in_=otmp[:])
                    if g == 3 and qt == 3 and 'DBG' in phases:
                        nc.scalar.dma_start(out=xt_dbg[:], in_=XT[:])
                        nc.scalar.dma_start(out=va_dbg[:], in_=VA[:])
                        nc.scalar.dma_start(out=ot_dbg[:], in_=OT[:])
                    # D completions for qt<3 are spread into ATT(3, qt+1)
                    # above; the last block's completion runs at the tail
                    if g == 3 and 'D' in phases and qt == 3:
                        # p-state warmup: harmless matmuls keep the PE busy
                        # through the final normalize/OT-shift chain
                        warm = psmm.tile([128, 512], F32, tag="mm", name="warm")
                        for _ in range(16):
                            nc.tensor.matmul(
                                warm[:], qkt[0:64, 0:128], qkt[0:64, 0:512],
                                start=True, stop=True,
                                tile_position=(0, 0))
                        for step in d_steps(3):
                            step()

    nc.compile()
    return nc


def _in_maps(x, W_attn, b_attn, W_proj, b_proj):
    import ml_dtypes
    BF = ml_dtypes.bfloat16
    ones64 = np.ones((128, 128), dtype=BF)

    in_maps = []
    for core in range(N_CORES):
        b = core // 2
        hg = core % 2
        sl = slice(hg * 512, (hg + 1) * 512)
        w_qkv = np.concatenate(
            [W_attn[:, 0:1024][:, sl], W_attn[:, 1024:2048][:, sl],
             W_attn[:, 2048:3072][:, sl]], axis=1)
        bq = b_attn[0:1024][sl]
        bk = b_attn[1024:2048][sl]
        bv = b_attn[2048:3072][sl]
        # b_qk [128, 8]: col half*4+g holds bias for W cols (half,g) chunk
        b_qk = np.stack(
            [bq[g * 128:(g + 1) * 128] for g in range(4)]
            + [bk[g * 128:(g + 1) * 128] for g in range(4)], axis=1)
        b_v = np.broadcast_to(bv, (128, 512)).copy()
        in_maps.append({
            "x": np.ascontiguousarray(x[b]).astype(BF),
            "w_qkv": np.ascontiguousarray(w_qkv).astype(BF),
            "w_proj": np.ascontiguousarray(W_proj[sl, :]).astype(BF),
            "b_qk": np.ascontiguousarray(b_qk.astype(np.float32)),
            "b_v": b_v.astype(np.float32),
            "ones64": ones64,
        })
    return in_maps


def kernel(x, W_attn, b_attn, W_proj, b_proj, _trace=False):
    from concourse.bass_utils import run_bass_kernel_spmd

    x = np.asarray(x, dtype=np.float32)
    W_attn = np.asarray(W_attn, dtype=np.float32)
    b_attn = np.asarray(b_attn, dtype=np.float32)
    W_proj = np.asarray(W_proj, dtype=np.float32)
    b_proj = np.asarray(b_proj, dtype=np.float32)

    if "nc" not in _CACHE:
        _CACHE["nc"] = _build()
    nc = _CACHE["nc"]

    in_maps = _in_maps(x, W_attn, b_attn, W_proj, b_proj)
    res = run_bass_kernel_spmd(nc, in_maps, list(range(N_CORES)), trace=_trace)
    B = x.shape[0]
    out = np.empty((B, T, C), np.float32)
    for b in range(B):
        out[b] = (res.results[2 * b]["out"].astype(np.float32)
                  + res.results[2 * b + 1]["out"].astype(np.float32) + b_proj)
    if _trace:
        _CACHE["last_result"] = res
    return out


# revision 49
# speedup vs baseline: 1.0046x; 1.0046x over previous
"""Causal self-attention kernel for Trainium2, 8 NeuronCores.

Problem: B=4, T=2048, C=1024, 16 heads, D=64 (fp32).
Sharding: core i handles batch b=i//2 and head-group hg=i%2 (8 heads each).
Each core computes qkv + attention + its partial projection; the host sums
the two head-group partials per batch and adds b_proj.

Dataflow (all matmul operands bf16, fp32 PSUM accumulate; rel err 3.6e-3):
  Wavefront (per 128-row t-chunk, DMA loads explicitly sequenced on the sync
    queue): x DMA (bf16, host-cast) -> PE transpose (1c/row) -> XT;
    V = x@Wv -> VA; QK(g=0) per 512-col strip.
  VA holds [ones x64 | V_h] per head, so each PV matmul (cost = N only)
    also produces the softmax denominators replicated on PSUM partitions
    0-63 while O lands on 64-127: normalization is a single DVE reciprocal
    + multiply (PSUM/SBUF operands may use different base partitions).
  ATT(g): scores (2 heads row-packed at tile_position 64), exp on Act,
    causal masks on gpsimd, PV; QK(g+1) pieces and (at g=3) the projection
    are interleaved between strips as PE filler, front-loaded at q-block
    boundaries where the next PV waits on the psO slot.  PV emission lags
    scores by 2 strip-pairs to avoid head-of-line blocking.
  Tail: warmup matmuls hold the PE p-state through the final normalize
    chain so the last projection block runs at full clock; outputs are
    stored bf16 (host upcasts and sums the partials).
"""

import numpy as np

N_CORES = 8
T = 2048
C = 1024
HL = 8          # heads per core
D = 64
KC = C // 128   # 8 contraction chunks
NT = T // 128   # 16 t-tiles
NQ = T // 512   # 4 q-tiles
VW = HL * 128   # 1024 v-aug cols per t-tile ([ones x64 | V] per head)

_CACHE = {}


def _build(phases=('A', 'QK', 'ATT', 'D')):
    from contextlib import ExitStack
    import concourse.bass as bass
    from concourse import bacc
    import concourse.mybir as mybir
    import concourse.tile as tile
    from concourse.masks import make_identity

    F32 = mybir.dt.float32
    BF16 = mybir.dt.bfloat16
    EXP = mybir.ActivationFunctionType.Exp
    ISGE = mybir.AluOpType.is_ge
    W15 = C + C // 2  # 1536

    nc = bacc.Bacc("TRN2", target_bir_lowering=False, debug=False,
                   num_devices=N_CORES)

    x_d = nc.dram_tensor("x", [T, C], BF16, kind="ExternalInput")
    wqkv_d = nc.dram_tensor("w_qkv", [C, W15], BF16, kind="ExternalInput")
    wproj_d = nc.dram_tensor("w_proj", [512, C], BF16, kind="ExternalInput")
    bqk_d = nc.dram_tensor("b_qk", [128, 8], F32, kind="ExternalInput")
    bv_d = nc.dram_tensor("b_v", [128, 512], F32, kind="ExternalInput")
    ones_d = nc.dram_tensor("ones64", [128, 128], BF16, kind="ExternalInput")
    out_d = nc.dram_tensor("out", [T, C], BF16, kind="ExternalOutput")
    if 'DBG' in phases:
        xt_dbg = nc.dram_tensor("xt_dbg", [128, KC * T], BF16, kind="ExternalOutput")
        va_dbg = nc.dram_tensor("va_dbg", [128, NT * VW], BF16, kind="ExternalOutput")
        qkt_dbg = nc.dram_tensor("qkt_dbg", [4, 128, 2 * T], BF16, kind="ExternalOutput")
        ot_dbg = nc.dram_tensor("ot_dbg", [128, 4 * T], BF16, kind="ExternalOutput")

    with tile.TileContext(nc) as tc, ExitStack() as ctx:
        # ---------- persistent pools ----------
        consts = ctx.enter_context(tc.tile_pool(name="consts", bufs=1))
        big = ctx.enter_context(tc.tile_pool(name="big", bufs=1))
        wqkp = ctx.enter_context(tc.tile_pool(name="wqk", bufs=2))
        qktp = ctx.enter_context(tc.tile_pool(name="qkt", bufs=2))

        ident = consts.tile([128, 128], BF16)
        make_identity(nc, ident[:])
        bqk_sb = consts.tile([128, 8], F32)
        nc.gpsimd.dma_start(out=bqk_sb[:], in_=bqk_d[:])
        ones_sb = consts.tile([128, 128], BF16)
        nc.gpsimd.dma_start(out=ones_sb[:], in_=ones_d[:])
        bv_sb = consts.tile([128, 512], F32)
        nc.gpsimd.dma_start(out=bv_sb[:], in_=bv_d[:])

        XT = big.tile([128, KC * T], BF16)        # 32 KB/part, x transposed
        VA = big.tile([128, NT * VW], BF16)       # 16.25 KB/part, v-aug
        OT = big.tile([128, 4 * T], BF16)         # 16 KB/part, attn out^T

        # ones-block columns of VA (cols [0:64) of each 128-col head group)
        va_ones = VA[:].rearrange("p (i c) -> p i c", c=128)[:, :, 0:64]
        nc.vector.tensor_copy(
            va_ones,
            ones_sb[:, 0:64].rearrange("p (x c) -> p x c", x=1)
            .broadcast_to([128, NT * HL, 64]))

        def emit_wqk(g, eng, halves=(0, 1)):
            wqk = wqkp.tile([128, 2 * KC * 128], BF16, tag="wqk", name=f"wqk{g}")
            for half in halves:
                eng.dma_start(
                    out=wqk[:, half * KC * 128:(half + 1) * KC * 128]
                        .rearrange("p (k m) -> p k m", k=KC),
                    in_=wqkv_d[:, half * 512 + g * 128: half * 512 + (g + 1) * 128]
                        .rearrange("(k p) m -> p k m", p=128),
                )
            return wqk

        def emit_wqk_half(g, wqk, half, eng):
            eng.dma_start(
                out=wqk[:, half * KC * 128:(half + 1) * KC * 128]
                    .rearrange("p (k m) -> p k m", k=KC),
                in_=wqkv_d[:, half * 512 + g * 128: half * 512 + (g + 1) * 128]
                    .rearrange("(k p) m -> p k m", p=128),
            )

        def emit_qk_chunk(g, wqk, qkt, nt4, psp):
            # QK(g) for t-strip nt4 (512 cols), both halves -> qkt bf16
            for half in (0, 1):
                pqk = psp.tile([128, 512], F32, tag="mm")
                for k in range(KC):
                    nc.tensor.matmul(
                        pqk[:],
                        wqk[:, half * KC * 128 + k * 128:
                               half * KC * 128 + (k + 1) * 128],
                        XT[:, k * T + nt4 * 512: k * T + (nt4 + 1) * 512],
                        start=(k == 0), stop=(k == KC - 1))
                nc.vector.tensor_scalar_add(
                    qkt[:, half * T + nt4 * 512: half * T + (nt4 + 1) * 512],
                    pqk[:],
                    bqk_sb[:, half * 4 + g: half * 4 + g + 1])

        def qk_chunk_steps(g, wqk, qkt, nt4, psp):
            # Same work as emit_qk_chunk but as a list of thunks, each
            # emitting a 2-matmul piece, so QK(g+1) can be spread between
            # attention strips (fills PE while Act works through the exps).
            steps = []
            state = {}
            def make_step(half, k2):
                def step():
                    if k2 == 0:
                        state[half] = psp.tile([128, 512], F32, tag="mm",
                                               name=f"pqk{g}_{nt4}_{half}")
                    pqk = state[half]
                    for k in (2 * k2, 2 * k2 + 1):
                        nc.tensor.matmul(
                            pqk[:],
                            wqk[:, half * KC * 128 + k * 128:
                                   half * KC * 128 + (k + 1) * 128],
                            XT[:, k * T + nt4 * 512: k * T + (nt4 + 1) * 512],
                            start=(k == 0), stop=(k == KC - 1))
                    if k2 == KC // 2 - 1:
                        nc.vector.tensor_scalar_add(
                            qkt[:, half * T + nt4 * 512:
                                half * T + (nt4 + 1) * 512],
                            pqk[:],
                            bqk_sb[:, half * 4 + g: half * 4 + g + 1])
                return step
            for half in (0, 1):
                for k2 in range(KC // 2):
                    steps.append(make_step(half, k2))
            return steps

        # ---------- wavefront: transpose x + V + QK(g=0) ----------
        # All DRAM loads go on the sync queue in an explicit order matched to
        # PE demand (the cost model's DMA pipe is serialized at ~350 B/ns):
        # x0ab, wv0, x1, wv1, x2, x3, wqk0a, x4, wqk0b, x5..x15, wqk1.
        # V(it) is split k0-3/k4-7 around transposes(it+1) so its first half
        # runs as soon as the first wv half lands.
        wqks = {}
        qkts = {}
        if 'A' in phases:
          with (
            tc.tile_pool(name="xnat", bufs=5) as xnat,
            tc.tile_pool(name="wv", bufs=1) as wvp,
            tc.tile_pool(name="psT", bufs=3, space="PSUM") as psT,
            tc.tile_pool(name="psA", bufs=2, space="PSUM") as psA,
          ):
            wv = wvp.tile([128, KC * 512], BF16)
            if 'QK' in phases:
                qkts[0] = qktp.tile([128, 2 * T], BF16, tag="qkt", name="qkt0")

            def emit_wv_half(h):
                nc.sync.dma_start(
                    out=wv[:, h * 4 * 512:(h + 1) * 4 * 512]
                        .rearrange("p (k m) -> p k m", k=4),
                    in_=wqkv_d[512 * h: 512 * (h + 1), 1024:1536]
                        .rearrange("(k p) m -> p k m", p=128),
                )

            pvs = {}

            def emit_v_part(it, part):
                if part == 0:
                    pvs[it] = psA.tile([128, 512], F32, tag="pv",
                                       name=f"pv{it}")
                pv = pvs[it]
                for k in range(4 * part, 4 * part + 4):
                    nc.tensor.matmul(
                        pv[:],
                        XT[:, k * T + it * 128: k * T + (it + 1) * 128],
                        wv[:, k * 512:(k + 1) * 512],
                        start=(k == 0), stop=(k == KC - 1))
                if part == 1:
                    va_dst = VA[:, it * VW:(it + 1) * VW].rearrange(
                        "p (h c) -> p h c", h=HL)[:, :, 64:128]
                    nc.vector.tensor_add(
                        va_dst,
                        pv[:].rearrange("p (h c) -> p h c", h=HL),
                        bv_sb[:].rearrange("p (h c) -> p h c", h=HL))

            for it in range(NT):
                xt = xnat.tile([128, C], BF16, tag="xn")
                if it == 0:
                    for hc in (0, 1):
                        nc.sync.dma_start(
                            out=xt[:, hc * 512:(hc + 1) * 512],
                            in_=x_d[0:128, hc * 512:(hc + 1) * 512])
                else:
                    nc.sync.dma_start(out=xt[:],
                                      in_=x_d[it * 128:(it + 1) * 128, :])
                if it == 1:
                    emit_wv_half(0)
                    emit_wv_half(1)
                elif 'QK' in phases and it == 6:
                    wqks[0] = emit_wqk(0, nc.sync, halves=(0,))
                elif 'QK' in phases and it == 7:
                    emit_wqk_half(0, wqks[0], 1, nc.sync)
                # V(it-1) part A: emitted after this iteration's weight DMAs
                # (its wv reads must follow the wv writes in program order)
                if it > 0:
                    emit_v_part(it - 1, 0)
                for c2 in (0, 1):
                    pt = psT.tile([128, 512], BF16, tag="tp")
                    for j in range(4):
                        nc.tensor.transpose(
                            pt[:, j * 128:(j + 1) * 128],
                            xt[:, (4 * c2 + j) * 128:(4 * c2 + j + 1) * 128],
                            ident[:])
                    dst = XT[:].rearrange("p (k t) -> p k t", k=KC)[
                        :, 4 * c2:4 * c2 + 4, it * 128:(it + 1) * 128]
                    src = pt[:].rearrange("p (k t) -> p k t", k=4)
                    if c2 == 0:
                        nc.vector.tensor_copy(dst, src)
                    else:
                        nc.scalar.copy(dst, src)
                if it > 0:
                    emit_v_part(it - 1, 1)
                if 'QK' in phases and it % 4 == 3 and it > 3:
                    emit_qk_chunk(0, wqks[0], qkts[0], it // 4 - 1, psA)
            emit_v_part(NT - 1, 0)
            emit_v_part(NT - 1, 1)
            if 'QK' in phases:
                emit_qk_chunk(0, wqks[0], qkts[0], NQ - 1, psA)
                wqks[1] = emit_wqk(1, nc.sync)

        # ---------- ATT(g) with QK(g+1) + D interleaved ----------
        if 'QK' in phases and 'ATT' in phases:
          with (
            tc.tile_pool(name="ptile", bufs=5) as ptp,
            tc.tile_pool(name="rsc", bufs=1) as rscp,
            tc.tile_pool(name="wp", bufs=1) as wpp,
            tc.tile_pool(name="stage", bufs=3) as stagep,
            tc.tile_pool(name="dpart", bufs=2) as dpartp,
            tc.tile_pool(name="psmm", bufs=2, space="PSUM") as psmm,
            tc.tile_pool(name="psS", bufs=2, space="PSUM") as psS,
            tc.tile_pool(name="psO", bufs=1, space="PSUM") as psO,
          ):
            wp = wpp.tile([128, 4 * C], BF16)
            if 'D' in phases:
                nc.gpsimd.dma_start(
                    out=wp[:].rearrange("p (g m) -> p g m", g=4),
                    in_=wproj_d[:].rearrange("(g p) m -> p g m", p=128),
                )
            for g in range(4):
                if g + 2 < 4:
                    wqks[g + 2] = emit_wqk(g + 2, nc.gpsimd)
                qkt = qkts[g]
                if g + 1 < 4:
                    qkts[g + 1] = qktp.tile([128, 2 * T], BF16, tag="qkt",
                                            name=f"qkt{g+1}")
                dparts = {}

                def d_partial_steps(qt):
                    # gg=0..2 projection partials for q-block qt: OT strips
                    # for head-groups 0-2 are complete once g=2 finished, so
                    # these can run as PE filler during ATT(3, qt) itself
                    steps = []
                    def make_step(it, n):
                        def step():
                            part = dpartp.tile([128, 512], F32,
                                               tag=f"part{it % 4}_{n}",
                                               name=f"part{it}_{n}")
                            dparts[(it, n)] = part
                            pp = psmm.tile([128, 512], F32, tag="mm",
                                           name=f"ppa{it}_{n}")
                            for gg in range(3):
                                nc.tensor.matmul(
                                    pp[:],
                                    OT[:, gg * T + it * 128: gg * T + (it + 1) * 128],
                                    wp[:, gg * C + n * 512: gg * C + (n + 1) * 512],
                                    start=(gg == 0), stop=(gg == 2))
                            nc.vector.tensor_copy(part[:], pp[:])
                        return step
                    for it in range(4 * qt, 4 * qt + 4):
                        for n in (0, 1):
                            steps.append(make_step(it, n))
                    return steps

                def d_steps(qt):
                    # whole-group projection (used for the tail block where
                    # the serial DVE adds of the split form would bind)
                    steps = []
                    state = {}
                    def make_step(it, n):
                        def step():
                            if n == 0:
                                state[it] = stagep.tile([128, C], BF16,
                                                        tag="stg",
                                                        name=f"stg{it}")
                            stage = state[it]
                            pp = psmm.tile([128, 512], F32, tag="mm",
                                           name=f"pp{it}_{n}")
                            for gg in range(4):
                                nc.tensor.matmul(
                                    pp[:],
                                    OT[:, gg * T + it * 128: gg * T + (it + 1) * 128],
                                    wp[:, gg * C + n * 512: gg * C + (n + 1) * 512],
                                    start=(gg == 0), stop=(gg == 3))
                            if n == 0:
                                nc.vector.tensor_copy(
                                    stage[:, 0:512], pp[:])
                            else:
                                nc.scalar.copy(
                                    stage[:, 512:1024], pp[:])
                            nc.sync.dma_start(
                                out=out_d[it * 128:(it + 1) * 128,
                                          n * 512:(n + 1) * 512],
                                in_=stage[:, n * 512:(n + 1) * 512])
                        return step
                    for it in range(4 * qt, 4 * qt + 4):
                        for n in (0, 1):
                            steps.append(make_step(it, n))
                    return steps

                def d_complete_steps(qt):
                    # gg=3 + add partial + store, per (it, n)
                    steps = []
                    state = {}
                    def make_step(it, n):
                        def step():
                            if n == 0:
                                state[it] = stagep.tile([128, C], BF16,
                                                        tag="stg",
                                                        name=f"stg{it}")
                            stage = state[it]
                            pp = psmm.tile([128, 512], F32, tag="mm",
                                           name=f"ppb{it}_{n}")
                            nc.tensor.matmul(
                                pp[:],
                                OT[:, 3 * T + it * 128: 3 * T + (it + 1) * 128],
                                wp[:, 3 * C + n * 512: 3 * C + (n + 1) * 512],
                                start=True, stop=True)
                            nc.vector.tensor_add(
                                stage[:, n * 512:(n + 1) * 512], pp[:],
                                dparts[(it, n)][:])
                            nc.sync.dma_start(
                                out=out_d[it * 128:(it + 1) * 128,
                                          n * 512:(n + 1) * 512],
                                in_=stage[:, n * 512:(n + 1) * 512])
                        return step
                    for it in range(4 * qt, 4 * qt + 4):
                        for n in (0, 1):
                            steps.append(make_step(it, n))
                    return steps

                if 'DBG' in phases:
                    nc.scalar.dma_start(out=qkt_dbg[g], in_=qkt[:])
                for qt in range(NQ):
                    qk_steps = []
                    if g + 1 < 4:
                        qk_steps = qk_chunk_steps(g + 1, wqks[g + 1],
                                                  qkts[g + 1], qt, psmm)
                    elif 'D' in phases:
                        if qt > 0:
                            qk_steps = d_complete_steps(qt - 1)
                        if qt < 3:
                            qk_steps = qk_steps + d_partial_steps(qt)
                    nsteps = len(qk_steps)
                    nstrips = 2 * (2 * qt + 2)
                    popped = [0]

                    def pop_qk(frac):
                        want = int(round(frac * nsteps))
                        while popped[0] < want:
                            qk_steps[popped[0]]()
                            popped[0] += 1
                    strip_i = [0]
                    psO0 = psO.tile([128, 512], F32, tag="o0")
                    psO1 = psO.tile([128, 512], F32, tag="o1")
                    psOh = [psO0, psO1]
                    if nsteps:
                        # front-load filler into the qt-boundary window where
                        # the PE would otherwise stall on the psO slot
                        pop_qk(0.25)

                    jlast = 4 * qt + 3
                    # q-restriction per diagonal delta: computed q-range
                    # [qoff, 512); bf16 matmuls run 1c/row at any N so the
                    # delta-3 strip computes only its 128-col triangle block.
                    QOFF = (0, 128, 256, 384)

                    def emit_pv(s, hi, ptile):
                        diag = s >= 2 * qt
                        h = 2 * g + hi
                        strip_i[0] += 1
                        if nsteps:
                            pop_qk(0.25 + 0.75 * strip_i[0] / nstrips)
                        for dd in (0, 1):
                            j = 2 * s + dd
                            qoff = QOFF[j - 4 * qt] if diag else 0
                            nc.tensor.matmul(
                                psOh[hi][:, qoff:512],
                                VA[:, j * VW + h * 128: j * VW + (h + 1) * 128],
                                ptile[:, dd * 512 + qoff:(dd + 1) * 512],
                                start=(j == 0), stop=(j == jlast))

                    # PV emission lags scores by 2 strip-pairs so the PE has
                    # score work queued while the previous q-block's psO is
                    # still being normalized (avoids head-of-line blocking).
                    pend = []
                    for s in range(2 * qt + 2):
                        diag = s >= 2 * qt
                        for hi in (0, 1):
                            psSt = psS.tile([128, 1024], F32, tag="psS")
                            for dd in (0, 1):
                                j = 2 * s + dd
                                qoff = QOFF[j - 4 * qt] if diag else 0
                                nc.tensor.matmul(
                                    psSt[:, dd * 512 + qoff:(dd + 1) * 512],
                                    qkt[64 * hi:64 * hi + 64,
                                        T + j * 128: T + (j + 1) * 128],
                                    qkt[64 * hi:64 * hi + 64,
                                        qt * 512 + qoff:(qt + 1) * 512],
                                    start=True, stop=True,
                                    tile_position=(64 * hi, 0))
                            ptile = ptp.tile([128, 1024], BF16, tag=f"pt{hi}")
                            if diag and s == 2 * qt + 1:
                                # deltas 2,3: cols [256:512] and [896:1024]
                                nc.scalar.activation(
                                    ptile[:, 256:512], psSt[:, 256:512],
                                    EXP, scale=0.125)
                                nc.scalar.activation(
                                    ptile[:, 896:1024], psSt[:, 896:1024],
                                    EXP, scale=0.125)
                            else:
                                nc.scalar.activation(ptile[:], psSt[:], EXP,
                                                     scale=0.125)
                            if diag:
                                for dd in (0, 1):
                                    delta = 2 * (s - 2 * qt) + dd
                                    # triangle block at cols [128*delta,+128):
                                    # keep where (q rel block) - k >= 0
                                    sl = slice(dd * 512 + 128 * delta,
                                               dd * 512 + 128 * delta + 128)
                                    nc.gpsimd.affine_select(
                                        out=ptile[:, sl], in_=ptile[:, sl],
                                        compare_op=ISGE, fill=0.0, base=0,
                                        pattern=[[1, 128]],
                                        channel_multiplier=-1)
                            pend.append((s, hi, ptile))
                            if len(pend) > 6:
                                emit_pv(*pend.pop(0))
                    for item in pend:
                        emit_pv(*item)
                    # normalize + store OT: sums sit replicated on psO
                    # partitions 0-63, O on 64-127 (PSUM/SBUF operands may
                    # use different base partitions)
                    for hi in (0, 1):
                        bc_sb = rscp.tile([64, 512], F32, tag=f"bc{hi}")
                        with nc.allow_low_precision(reason="softmax recip"):
                            nc.vector.reciprocal(bc_sb[:], psOh[hi][0:64, :])
                        # DVE writes either OT half directly: output/PSUM-input
                        # partition bases may differ from the SBUF input's
                        nc.vector.tensor_mul(
                            OT[64 * hi:64 * hi + 64,
                               g * T + qt * 512: g * T + (qt + 1) * 512],
                            psOh[hi][64:128, :], bc_sb[:])
                    if g == 3 and qt == 3 and 'DBG' in phases:
                        nc.scalar.dma_start(out=xt_dbg[:], in_=XT[:])
                        nc.scalar.dma_start(out=va_dbg[:], in_=VA[:])
                        nc.scalar.dma_start(out=ot_dbg[:], in_=OT[:])
                    # D completions for qt<3 are spread into ATT(3, qt+1)
                    # above; the last block's completion runs at the tail
                    if g == 3 and 'D' in phases and qt == 3:
                        # p-state warmup: harmless matmuls keep the PE busy
                        # through the final normalize/OT-shift chain
                        warm = psmm.tile([128, 512], F32, tag="mm", name="warm")
                        for _ in range(12):
                            nc.tensor.matmul(
                                warm[:], qkt[0:64, 0:128], qkt[0:64, 0:512],
                                start=True, stop=True,
                                tile_position=(0, 0))
                        for step in d_steps(3):
                            step()

    nc.compile()
    return nc


def _in_maps(x, W_attn, b_attn, W_proj, b_proj):
    import ml_dtypes
    BF = ml_dtypes.bfloat16
    ones64 = np.ones((128, 128), dtype=BF)

    in_maps = []
    for core in range(N_CORES):
        b = core // 2
        hg = core % 2
        sl = slice(hg * 512, (hg + 1) * 512)
        w_qkv = np.concatenate(
            [W_attn[:, 0:1024][:, sl], W_attn[:, 1024:2048][:, sl],
             W_attn[:, 2048:3072][:, sl]], axis=1)
        bq = b_attn[0:1024][sl]
        bk = b_attn[1024:2048][sl]
        bv = b_attn[2048:3072][sl]
        # b_qk [128, 8]: col half*4+g holds bias for W cols (half,g) chunk
        b_qk = np.stack(
            [bq[g * 128:(g + 1) * 128] for g in range(4)]
            + [bk[g * 128:(g + 1) * 128] for g in range(4)], axis=1)
        b_v = np.broadcast_to(bv, (128, 512)).copy()
        in_maps.append({
            "x": np.ascontiguousarray(x[b]).astype(BF),
            "w_qkv": np.ascontiguousarray(w_qkv).astype(BF),
            "w_proj": np.ascontiguousarray(W_proj[sl, :]).astype(BF),
            "b_qk": np.ascontiguousarray(b_qk.astype(np.float32)),
            "b_v": b_v.astype(np.float32),
            "ones64": ones64,
        })
    return in_maps


def kernel(x, W_attn, b_attn, W_proj, b_proj, _trace=False):
    from concourse.bass_utils import run_bass_kernel_spmd

    x = np.asarray(x, dtype=np.float32)
    W_attn = np.asarray(W_attn, dtype=np.float32)
    b_attn = np.asarray(b_attn, dtype=np.float32)
    W_proj = np.asarray(W_proj, dtype=np.float32)
    b_proj = np.asarray(b_proj, dtype=np.float32)

    if "nc" not in _CACHE:
        _CACHE["nc"] = _build()
    nc = _CACHE["nc"]

    in_maps = _in_maps(x, W_attn, b_attn, W_proj, b_proj)
    res = run_bass_kernel_spmd(nc, in_maps, list(range(N_CORES)), trace=_trace)
    B = x.shape[0]
    out = np.empty((B, T, C), np.float32)
    for b in range(B):
        out[b] = (res.results[2 * b]["out"].astype(np.float32)
                  + res.results[2 * b + 1]["out"].astype(np.float32) + b_proj)
    if _trace:
        _CACHE["last_result"] = res
    return out


# revision 55
# speedup vs baseline: 1.0066x; 1.0020x over previous
"""Causal self-attention kernel for Trainium2, 8 NeuronCores.

Problem: B=4, T=2048, C=1024, 16 heads, D=64 (fp32).
Sharding: core i handles batch b=i//2 and head-group hg=i%2 (8 heads each).
Each core computes qkv + attention + its partial projection; the host sums
the two head-group partials per batch and adds b_proj.

Dataflow (all matmul operands bf16, fp32 PSUM accumulate; rel err 3.6e-3):
  Wavefront (per 128-row t-chunk, DMA loads explicitly sequenced on the sync
    queue): x DMA (bf16, host-cast) -> PE transpose (1c/row) -> XT;
    V = x@Wv -> VA; QK(g=0) per 512-col strip.
  VA holds [ones x64 | V_h] per head, so each PV matmul (cost = N only)
    also produces the softmax denominators replicated on PSUM partitions
    0-63 while O lands on 64-127: normalization is a single DVE reciprocal
    + multiply (PSUM/SBUF operands may use different base partitions).
  ATT(g): scores (2 heads row-packed at tile_position 64), exp on Act,
    causal masks on gpsimd, PV; QK(g+1) pieces and (at g=3) the projection
    are interleaved between strips as PE filler, front-loaded at q-block
    boundaries where the next PV waits on the psO slot.  PV emission lags
    scores by 2 strip-pairs to avoid head-of-line blocking.
  Tail: warmup matmuls hold the PE p-state through the final normalize
    chain so the last projection block runs at full clock; outputs are
    stored bf16 (host upcasts and sums the partials).
"""

import numpy as np

N_CORES = 8
T = 2048
C = 1024
HL = 8          # heads per core
D = 64
KC = C // 128   # 8 contraction chunks
NT = T // 128   # 16 t-tiles
NQ = T // 512   # 4 q-tiles
VW = HL * 128   # 1024 v-aug cols per t-tile ([ones x64 | V] per head)

_CACHE = {}


def _build(phases=('A', 'QK', 'ATT', 'D')):
    from contextlib import ExitStack
    import concourse.bass as bass
    from concourse import bacc
    import concourse.mybir as mybir
    import concourse.tile as tile
    from concourse.masks import make_identity

    F32 = mybir.dt.float32
    BF16 = mybir.dt.bfloat16
    EXP = mybir.ActivationFunctionType.Exp
    ISGE = mybir.AluOpType.is_ge
    W15 = C + C // 2  # 1536

    nc = bacc.Bacc("TRN2", target_bir_lowering=False, debug=False,
                   num_devices=N_CORES)

    x_d = nc.dram_tensor("x", [T, C], BF16, kind="ExternalInput")
    wqkv_d = nc.dram_tensor("w_qkv", [C, W15], BF16, kind="ExternalInput")
    wproj_d = nc.dram_tensor("w_proj", [512, C], BF16, kind="ExternalInput")
    bqk_d = nc.dram_tensor("b_qk", [128, 8], F32, kind="ExternalInput")
    bv_d = nc.dram_tensor("b_v", [128, 512], F32, kind="ExternalInput")
    ones_d = nc.dram_tensor("ones64", [128, 128], BF16, kind="ExternalInput")
    out_d = nc.dram_tensor("out", [T, C], BF16, kind="ExternalOutput")
    if 'DBG' in phases:
        xt_dbg = nc.dram_tensor("xt_dbg", [128, KC * T], BF16, kind="ExternalOutput")
        va_dbg = nc.dram_tensor("va_dbg", [128, NT * VW], BF16, kind="ExternalOutput")
        qkt_dbg = nc.dram_tensor("qkt_dbg", [4, 128, 2 * T], BF16, kind="ExternalOutput")
        ot_dbg = nc.dram_tensor("ot_dbg", [128, 4 * T], BF16, kind="ExternalOutput")

    with tile.TileContext(nc) as tc, ExitStack() as ctx:
        # ---------- persistent pools ----------
        consts = ctx.enter_context(tc.tile_pool(name="consts", bufs=1))
        big = ctx.enter_context(tc.tile_pool(name="big", bufs=1))
        wqkp = ctx.enter_context(tc.tile_pool(name="wqk", bufs=2))
        qktp = ctx.enter_context(tc.tile_pool(name="qkt", bufs=2))

        ident = consts.tile([128, 128], BF16)
        make_identity(nc, ident[:])
        bqk_sb = consts.tile([128, 8], F32)
        nc.gpsimd.dma_start(out=bqk_sb[:], in_=bqk_d[:])
        ones_sb = consts.tile([128, 128], BF16)
        nc.gpsimd.dma_start(out=ones_sb[:], in_=ones_d[:])
        bv_sb = consts.tile([128, 512], F32)
        nc.gpsimd.dma_start(out=bv_sb[:], in_=bv_d[:])

        XT = big.tile([128, KC * T], BF16)        # 32 KB/part, x transposed
        VA = big.tile([128, NT * VW], BF16)       # 16.25 KB/part, v-aug
        OT = big.tile([128, 4 * T], BF16)         # 16 KB/part, attn out^T

        # ones-block columns of VA (cols [0:64) of each 128-col head group)
        va_ones = VA[:].rearrange("p (i c) -> p i c", c=128)[:, :, 0:64]
        nc.vector.tensor_copy(
            va_ones,
            ones_sb[:, 0:64].rearrange("p (x c) -> p x c", x=1)
            .broadcast_to([128, NT * HL, 64]))

        def emit_wqk(g, eng, halves=(0, 1)):
            wqk = wqkp.tile([128, 2 * KC * 128], BF16, tag="wqk", name=f"wqk{g}")
            for half in halves:
                eng.dma_start(
                    out=wqk[:, half * KC * 128:(half + 1) * KC * 128]
                        .rearrange("p (k m) -> p k m", k=KC),
                    in_=wqkv_d[:, half * 512 + g * 128: half * 512 + (g + 1) * 128]
                        .rearrange("(k p) m -> p k m", p=128),
                )
            return wqk

        def emit_wqk_half(g, wqk, half, eng):
            eng.dma_start(
                out=wqk[:, half * KC * 128:(half + 1) * KC * 128]
                    .rearrange("p (k m) -> p k m", k=KC),
                in_=wqkv_d[:, half * 512 + g * 128: half * 512 + (g + 1) * 128]
                    .rearrange("(k p) m -> p k m", p=128),
            )

        def emit_qk_chunk(g, wqk, qkt, nt4, psp):
            # QK(g) for t-strip nt4 (512 cols), both halves -> qkt bf16
            for half in (0, 1):
                pqk = psp.tile([128, 512], F32, tag="mm")
                for k in range(KC):
                    nc.tensor.matmul(
                        pqk[:],
                        wqk[:, half * KC * 128 + k * 128:
                               half * KC * 128 + (k + 1) * 128],
                        XT[:, k * T + nt4 * 512: k * T + (nt4 + 1) * 512],
                        start=(k == 0), stop=(k == KC - 1))
                nc.vector.tensor_scalar_add(
                    qkt[:, half * T + nt4 * 512: half * T + (nt4 + 1) * 512],
                    pqk[:],
                    bqk_sb[:, half * 4 + g: half * 4 + g + 1])

        def qk_chunk_steps(g, wqk, qkt, nt4, psp):
            # Same work as emit_qk_chunk but as a list of thunks, each
            # emitting a 2-matmul piece, so QK(g+1) can be spread between
            # attention strips (fills PE while Act works through the exps).
            steps = []
            state = {}
            def make_step(half, k2):
                def step():
                    if k2 == 0:
                        state[half] = psp.tile([128, 512], F32, tag="mm",
                                               name=f"pqk{g}_{nt4}_{half}")
                    pqk = state[half]
                    for k in (2 * k2, 2 * k2 + 1):
                        nc.tensor.matmul(
                            pqk[:],
                            wqk[:, half * KC * 128 + k * 128:
                                   half * KC * 128 + (k + 1) * 128],
                            XT[:, k * T + nt4 * 512: k * T + (nt4 + 1) * 512],
                            start=(k == 0), stop=(k == KC - 1))
                    if k2 == KC // 2 - 1:
                        nc.vector.tensor_scalar_add(
                            qkt[:, half * T + nt4 * 512:
                                half * T + (nt4 + 1) * 512],
                            pqk[:],
                            bqk_sb[:, half * 4 + g: half * 4 + g + 1])
                return step
            for half in (0, 1):
                for k2 in range(KC // 2):
                    steps.append(make_step(half, k2))
            return steps

        # ---------- wavefront: transpose x + V + QK(g=0) ----------
        # All DRAM loads go on the sync queue in an explicit order matched to
        # PE demand (the cost model's DMA pipe is serialized at ~350 B/ns):
        # x0ab, wv0, x1, wv1, x2, x3, wqk0a, x4, wqk0b, x5..x15, wqk1.
        # V(it) is split k0-3/k4-7 around transposes(it+1) so its first half
        # runs as soon as the first wv half lands.
        wqks = {}
        qkts = {}
        if 'A' in phases:
          with (
            tc.tile_pool(name="xnat", bufs=5) as xnat,
            tc.tile_pool(name="wv", bufs=1) as wvp,
            tc.tile_pool(name="psT", bufs=2, space="PSUM") as psT,
            tc.tile_pool(name="psA", bufs=3, space="PSUM") as psA,
          ):
            wv = wvp.tile([128, KC * 512], BF16)
            if 'QK' in phases:
                qkts[0] = qktp.tile([128, 2 * T], BF16, tag="qkt", name="qkt0")

            def emit_wv_half(h):
                nc.sync.dma_start(
                    out=wv[:, h * 4 * 512:(h + 1) * 4 * 512]
                        .rearrange("p (k m) -> p k m", k=4),
                    in_=wqkv_d[512 * h: 512 * (h + 1), 1024:1536]
                        .rearrange("(k p) m -> p k m", p=128),
                )

            pvs = {}

            def emit_v_part(it, part):
                if part == 0:
                    pvs[it] = psA.tile([128, 512], F32, tag="pv",
                                       name=f"pv{it}")
                pv = pvs[it]
                for k in range(4 * part, 4 * part + 4):
                    nc.tensor.matmul(
                        pv[:],
                        XT[:, k * T + it * 128: k * T + (it + 1) * 128],
                        wv[:, k * 512:(k + 1) * 512],
                        start=(k == 0), stop=(k == KC - 1))
                if part == 1:
                    va_dst = VA[:, it * VW:(it + 1) * VW].rearrange(
                        "p (h c) -> p h c", h=HL)[:, :, 64:128]
                    nc.vector.tensor_add(
                        va_dst,
                        pv[:].rearrange("p (h c) -> p h c", h=HL),
                        bv_sb[:].rearrange("p (h c) -> p h c", h=HL))

            for it in range(NT):
                xt = xnat.tile([128, C], BF16, tag="xn")
                if it == 0:
                    for hc in (0, 1):
                        nc.sync.dma_start(
                            out=xt[:, hc * 512:(hc + 1) * 512],
                            in_=x_d[0:128, hc * 512:(hc + 1) * 512])
                else:
                    nc.sync.dma_start(out=xt[:],
                                      in_=x_d[it * 128:(it + 1) * 128, :])
                if it == 1:
                    emit_wv_half(0)
                    emit_wv_half(1)
                elif 'QK' in phases and it == 6:
                    wqks[0] = emit_wqk(0, nc.sync, halves=(0,))
                elif 'QK' in phases and it == 7:
                    emit_wqk_half(0, wqks[0], 1, nc.sync)
                # V(it-1) part A: emitted after this iteration's weight DMAs
                # (its wv reads must follow the wv writes in program order)
                if it > 0:
                    emit_v_part(it - 1, 0)
                for c2 in (0, 1):
                    pt = psT.tile([128, 512], BF16, tag="tp")
                    for j in range(4):
                        nc.tensor.transpose(
                            pt[:, j * 128:(j + 1) * 128],
                            xt[:, (4 * c2 + j) * 128:(4 * c2 + j + 1) * 128],
                            ident[:])
                    dst = XT[:].rearrange("p (k t) -> p k t", k=KC)[
                        :, 4 * c2:4 * c2 + 4, it * 128:(it + 1) * 128]
                    src = pt[:].rearrange("p (k t) -> p k t", k=4)
                    if c2 == 0:
                        nc.vector.tensor_copy(dst, src)
                    else:
                        nc.scalar.copy(dst, src)
                if it > 0:
                    emit_v_part(it - 1, 1)
                if 'QK' in phases and it % 4 == 3 and it > 3:
                    emit_qk_chunk(0, wqks[0], qkts[0], it // 4 - 1, psA)
            emit_v_part(NT - 1, 0)
            emit_v_part(NT - 1, 1)
            if 'QK' in phases:
                emit_qk_chunk(0, wqks[0], qkts[0], NQ - 1, psA)
                wqks[1] = emit_wqk(1, nc.sync)

        # ---------- ATT(g) with QK(g+1) + D interleaved ----------
        if 'QK' in phases and 'ATT' in phases:
          with (
            tc.tile_pool(name="ptile", bufs=5) as ptp,
            tc.tile_pool(name="rsc", bufs=1) as rscp,
            tc.tile_pool(name="wp", bufs=1) as wpp,
            tc.tile_pool(name="stage", bufs=4) as stagep,
            tc.tile_pool(name="dpart", bufs=2) as dpartp,
            tc.tile_pool(name="psmm", bufs=2, space="PSUM") as psmm,
            tc.tile_pool(name="psS", bufs=2, space="PSUM") as psS,
            tc.tile_pool(name="psO", bufs=1, space="PSUM") as psO,
          ):
            wp = wpp.tile([128, 4 * C], BF16)
            if 'D' in phases:
                nc.gpsimd.dma_start(
                    out=wp[:].rearrange("p (g m) -> p g m", g=4),
                    in_=wproj_d[:].rearrange("(g p) m -> p g m", p=128),
                )
            for g in range(4):
                if g + 2 < 4:
                    wqks[g + 2] = emit_wqk(g + 2, nc.gpsimd)
                qkt = qkts[g]
                if g + 1 < 4:
                    qkts[g + 1] = qktp.tile([128, 2 * T], BF16, tag="qkt",
                                            name=f"qkt{g+1}")
                dparts = {}

                def d_partial_steps(qt):
                    # gg=0..2 projection partials for q-block qt: OT strips
                    # for head-groups 0-2 are complete once g=2 finished, so
                    # these can run as PE filler during ATT(3, qt) itself
                    steps = []
                    def make_step(it, n):
                        def step():
                            part = dpartp.tile([128, 512], F32,
                                               tag=f"part{it % 4}_{n}",
                                               name=f"part{it}_{n}")
                            dparts[(it, n)] = part
                            pp = psmm.tile([128, 512], F32, tag="mm",
                                           name=f"ppa{it}_{n}")
                            for gg in range(3):
                                nc.tensor.matmul(
                                    pp[:],
                                    OT[:, gg * T + it * 128: gg * T + (it + 1) * 128],
                                    wp[:, gg * C + n * 512: gg * C + (n + 1) * 512],
                                    start=(gg == 0), stop=(gg == 2))
                            nc.vector.tensor_copy(part[:], pp[:])
                        return step
                    for it in range(4 * qt, 4 * qt + 4):
                        for n in (0, 1):
                            steps.append(make_step(it, n))
                    return steps

                def d_steps(qt):
                    # whole-group projection (used for the tail block where
                    # the serial DVE adds of the split form would bind)
                    steps = []
                    state = {}
                    def make_step(it, n):
                        def step():
                            if n == 0:
                                state[it] = stagep.tile([128, C], BF16,
                                                        tag="stg",
                                                        name=f"stg{it}")
                            stage = state[it]
                            pp = psmm.tile([128, 512], F32, tag="mm",
                                           name=f"pp{it}_{n}")
                            for gg in range(4):
                                nc.tensor.matmul(
                                    pp[:],
                                    OT[:, gg * T + it * 128: gg * T + (it + 1) * 128],
                                    wp[:, gg * C + n * 512: gg * C + (n + 1) * 512],
                                    start=(gg == 0), stop=(gg == 3))
                            if n == 0:
                                nc.vector.tensor_copy(
                                    stage[:, 0:512], pp[:])
                            else:
                                nc.scalar.copy(
                                    stage[:, 512:1024], pp[:])
                            nc.sync.dma_start(
                                out=out_d[it * 128:(it + 1) * 128,
                                          n * 512:(n + 1) * 512],
                                in_=stage[:, n * 512:(n + 1) * 512])
                        return step
                    for it in range(4 * qt, 4 * qt + 4):
                        for n in (0, 1):
                            steps.append(make_step(it, n))
                    return steps

                def d_complete_steps(qt):
                    # gg=3 + add partial + store, per (it, n)
                    steps = []
                    state = {}
                    def make_step(it, n):
                        def step():
                            if n == 0:
                                state[it] = stagep.tile([128, C], BF16,
                                                        tag="stg",
                                                        name=f"stg{it}")
                            stage = state[it]
                            pp = psmm.tile([128, 512], F32, tag="mm",
                                           name=f"ppb{it}_{n}")
                            nc.tensor.matmul(
                                pp[:],
                                OT[:, 3 * T + it * 128: 3 * T + (it + 1) * 128],
                                wp[:, 3 * C + n * 512: 3 * C + (n + 1) * 512],
                                start=True, stop=True)
                            nc.vector.tensor_add(
                                stage[:, n * 512:(n + 1) * 512], pp[:],
                                dparts[(it, n)][:])
                            nc.sync.dma_start(
                                out=out_d[it * 128:(it + 1) * 128,
                                          n * 512:(n + 1) * 512],
                                in_=stage[:, n * 512:(n + 1) * 512])
                        return step
                    for it in range(4 * qt, 4 * qt + 4):
                        for n in (0, 1):
                            steps.append(make_step(it, n))
                    return steps

                if 'DBG' in phases:
                    nc.scalar.dma_start(out=qkt_dbg[g], in_=qkt[:])
                for qt in range(NQ):
                    qk_steps = []
                    if g + 1 < 4:
                        qk_steps = qk_chunk_steps(g + 1, wqks[g + 1],
                                                  qkts[g + 1], qt, psmm)
                    elif 'D' in phases:
                        if qt > 0:
                            qk_steps = d_complete_steps(qt - 1)
                        if qt < 3:
                            qk_steps = qk_steps + d_partial_steps(qt)
                    nsteps = len(qk_steps)
                    nstrips = 2 * (2 * qt + 2)
                    popped = [0]

                    def pop_qk(frac):
                        want = int(round(frac * nsteps))
                        while popped[0] < want:
                            qk_steps[popped[0]]()
                            popped[0] += 1
                    strip_i = [0]
                    psO0 = psO.tile([128, 512], F32, tag="o0")
                    psO1 = psO.tile([128, 512], F32, tag="o1")
                    psOh = [psO0, psO1]
                    if nsteps:
                        # front-load filler into the qt-boundary window where
                        # the PE would otherwise stall on the psO slot
                        pop_qk(0.25)

                    jlast = 4 * qt + 3
                    # q-restriction per diagonal delta: computed q-range
                    # [qoff, 512); bf16 matmuls run 1c/row at any N so the
                    # delta-3 strip computes only its 128-col triangle block.
                    QOFF = (0, 128, 256, 384)

                    def emit_pv(s, hi, ptile):
                        diag = s >= 2 * qt
                        h = 2 * g + hi
                        strip_i[0] += 1
                        if nsteps:
                            pop_qk(0.25 + 0.75 * strip_i[0] / nstrips)
                        for dd in (0, 1):
                            j = 2 * s + dd
                            qoff = QOFF[j - 4 * qt] if diag else 0
                            nc.tensor.matmul(
                                psOh[hi][:, qoff:512],
                                VA[:, j * VW + h * 128: j * VW + (h + 1) * 128],
                                ptile[:, dd * 512 + qoff:(dd + 1) * 512],
                                start=(j == 0), stop=(j == jlast))

                    # PV emission lags scores by 2 strip-pairs so the PE has
                    # score work queued while the previous q-block's psO is
                    # still being normalized (avoids head-of-line blocking).
                    pend = []
                    for s in range(2 * qt + 2):
                        diag = s >= 2 * qt
                        for hi in (0, 1):
                            psSt = psS.tile([128, 1024], F32, tag="psS")
                            for dd in (0, 1):
                                j = 2 * s + dd
                                qoff = QOFF[j - 4 * qt] if diag else 0
                                nc.tensor.matmul(
                                    psSt[:, dd * 512 + qoff:(dd + 1) * 512],
                                    qkt[64 * hi:64 * hi + 64,
                                        T + j * 128: T + (j + 1) * 128],
                                    qkt[64 * hi:64 * hi + 64,
                                        qt * 512 + qoff:(qt + 1) * 512],
                                    start=True, stop=True,
                                    tile_position=(64 * hi, 0))
                            ptile = ptp.tile([128, 1024], BF16, tag=f"pt{hi}")
                            if diag and s == 2 * qt + 1:
                                # deltas 2,3: cols [256:512] and [896:1024]
                                nc.scalar.activation(
                                    ptile[:, 256:512], psSt[:, 256:512],
                                    EXP, scale=0.125)
                                nc.scalar.activation(
                                    ptile[:, 896:1024], psSt[:, 896:1024],
                                    EXP, scale=0.125)
                            else:
                                nc.scalar.activation(ptile[:], psSt[:], EXP,
                                                     scale=0.125)
                            if diag:
                                for dd in (0, 1):
                                    delta = 2 * (s - 2 * qt) + dd
                                    # triangle block at cols [128*delta,+128):
                                    # keep where (q rel block) - k >= 0
                                    sl = slice(dd * 512 + 128 * delta,
                                               dd * 512 + 128 * delta + 128)
                                    nc.gpsimd.affine_select(
                                        out=ptile[:, sl], in_=ptile[:, sl],
                                        compare_op=ISGE, fill=0.0, base=0,
                                        pattern=[[1, 128]],
                                        channel_multiplier=-1)
                            pend.append((s, hi, ptile))
                            if len(pend) > 7:
                                emit_pv(*pend.pop(0))
                    # drain hi=0 PVs first: recip0 (which gates the next
                    # q-block's first PV via the psO0 slot) depends only on
                    # the last hi=0 PV, so it can start while hi=1 PVs run
                    for item in sorted(pend, key=lambda t: (t[1], t[0])):
                        emit_pv(*item)
                    # normalize + store OT: sums sit replicated on psO
                    # partitions 0-63, O on 64-127 (PSUM/SBUF operands may
                    # use different base partitions)
                    for hi in (0, 1):
                        bc_sb = rscp.tile([64, 512], F32, tag=f"bc{hi}")
                        with nc.allow_low_precision(reason="softmax recip"):
                            nc.vector.reciprocal(bc_sb[:], psOh[hi][0:64, :])
                        # DVE writes either OT half directly: output/PSUM-input
                        # partition bases may differ from the SBUF input's
                        nc.vector.tensor_mul(
                            OT[64 * hi:64 * hi + 64,
                               g * T + qt * 512: g * T + (qt + 1) * 512],
                            psOh[hi][64:128, :], bc_sb[:])
                    if g == 3 and qt == 3 and 'DBG' in phases:
                        nc.scalar.dma_start(out=xt_dbg[:], in_=XT[:])
                        nc.scalar.dma_start(out=va_dbg[:], in_=VA[:])
                        nc.scalar.dma_start(out=ot_dbg[:], in_=OT[:])
                    # D completions for qt<3 are spread into ATT(3, qt+1)
                    # above; the last block's completion runs at the tail
                    if g == 3 and 'D' in phases and qt == 3:
                        # p-state warmup: harmless matmuls keep the PE busy
                        # through the final normalize/OT-shift chain
                        warm = psmm.tile([128, 512], F32, tag="mm", name="warm")
                        for _ in range(12):
                            nc.tensor.matmul(
                                warm[:], qkt[0:64, 0:128], qkt[0:64, 0:512],
                                start=True, stop=True,
                                tile_position=(0, 0))
                        for step in d_steps(3):
                            step()

    nc.compile()
    return nc


def _in_maps(x, W_attn, b_attn, W_proj, b_proj):
    import ml_dtypes
    BF = ml_dtypes.bfloat16
    ones64 = np.ones((128, 128), dtype=BF)

    in_maps = []
    for core in range(N_CORES):
        b = core // 2
        hg = core % 2
        sl = slice(hg * 512, (hg + 1) * 512)
        w_qkv = np.concatenate(
            [W_attn[:, 0:1024][:, sl], W_attn[:, 1024:2048][:, sl],
             W_attn[:, 2048:3072][:, sl]], axis=1)
        bq = b_attn[0:1024][sl]
        bk = b_attn[1024:2048][sl]
        bv = b_attn[2048:3072][sl]
        # b_qk [128, 8]: col half*4+g holds bias for W cols (half,g) chunk
        b_qk = np.stack(
            [bq[g * 128:(g + 1) * 128] for g in range(4)]
            + [bk[g * 128:(g + 1) * 128] for g in range(4)], axis=1)
        b_v = np.broadcast_to(bv, (128, 512)).copy()
        in_maps.append({
            "x": np.ascontiguousarray(x[b]).astype(BF),
            "w_qkv": np.ascontiguousarray(w_qkv).astype(BF),
            "w_proj": np.ascontiguousarray(W_proj[sl, :]).astype(BF),
            "b_qk": np.ascontiguousarray(b_qk.astype(np.float32)),
            "b_v": b_v.astype(np.float32),
            "ones64": ones64,
        })
    return in_maps


def kernel(x, W_attn, b_attn, W_proj, b_proj, _trace=False):
    from concourse.bass_utils import run_bass_kernel_spmd

    x = np.asarray(x, dtype=np.float32)
    W_attn = np.asarray(W_attn, dtype=np.float32)
    b_attn = np.asarray(b_attn, dtype=np.float32)
    W_proj = np.asarray(W_proj, dtype=np.float32)
    b_proj = np.asarray(b_proj, dtype=np.float32)

    if "nc" not in _CACHE:
        _CACHE["nc"] = _build()
    nc = _CACHE["nc"]

    in_maps = _in_maps(x, W_attn, b_attn, W_proj, b_proj)
    res = run_bass_kernel_spmd(nc, in_maps, list(range(N_CORES)), trace=_trace)
    B = x.shape[0]
    out = np.empty((B, T, C), np.float32)
    for b in range(B):
        out[b] = (res.results[2 * b]["out"].astype(np.float32)
                  + res.results[2 * b + 1]["out"].astype(np.float32) + b_proj)
    if _trace:
        _CACHE["last_result"] = res
    return out
